# revision 13
# baseline (speedup 1.0000x reference)
"""MultiHeadClassifier (MoE routing) Trainium2 kernel — int8-transfer edition.

Problem: B=65536 samples of dim D=1024, each routed by task_id to one of
T=16 two-layer heads (D->H=128 relu -> C=10). Host routes samples to their
head (only ~17 GFLOP of useful work), data-parallel with 2 tasks per core
across 8 cores.

Per-core budget: PE needs ~32us (bf16 streaming of ~8448 samples x 1024
contraction + layer 2); x as bf16 is 16.9MB of HBM (~50us, DMA-bound); as
int8 it is 8.65MB (~28us). The catch is the int8->bf16 upconversion: DVE
copies at 2 elem/cyc (245 G/s), ScalarE at 1 elem/cyc @1.2GHz, and casting
all 8 d-chunks would consume both engines completely. The budget closes by:
  - d-chunks 6,7 travel as raw bf16 (zero cast cost; HBM pays +2.2MB,
    pre-scaled by 127/4sigma on host to match the int8-folded W1)
  - d-chunks 0..4 are DVE-cast per sub, d-chunk 5 ScalarE-cast; relu+b1
    is split ScalarE-activation / DVE-scalar_tensor_tensor by a greedy
    load balancer
  - layer-2 matmuls col-group packed: the <=4 subs of an m-unit target
    partitions 32j..32j+9 of ONE psum bank via tile_position=(0,32j); one
    ScalarE copy drains a whole unit and the matmuls overlap in the array
  - weights arrive as one blob DMA per slot (b1 bitcast into bf16 cols)
  - b2 is added on the host during unshard

DMA delivery is matched to the compute schedule's demand curve across all
three rings (each ~240-340 GB/s): the int8 stream ships per-(slot,unit) —
units 0-1 on the sync HWDGE ring, units 2+ on the SWDGE ring (raw SWDGE
runs near line rate; only *casting* SWDGE DMAs crawl) — and the bf16
stream ships on the scalar ring interleaved with the weight blobs in
first-need order. Every stream is packed *flat per partition* in
sub-major order so each DMA is one contiguous multi-KB run per partition
and each engine cast is an exact contiguous 2D op. All x tiles stay
resident in SBUF so no DMA instruction ever waits (a waiting dma_start
blocks its issuing engine's queue). m-units per slot grow
[512, 1024, 1024, rest]. PE warmup fillers ride through the ~7us NEFF
preamble so real matmuls start warm at 2.4 GHz. int8 rel err ~9.5e-3
(gate 2e-2): clip 4sigma, scale folded into W1 on host.
"""

import sys

import numpy as np

for _p in ("/opt/trn_rl_repo", "/root/.axon_site/_ro/trn_rl_repo"):
    if _p not in sys.path:
        sys.path.append(_p)

import concourse.bacc as bacc
import concourse.mybir as mybir
from concourse.bass_utils import run_bass_kernel_spmd
from concourse.tile import TileContext

B, D, T, H, C = 65536, 1024, 16, 128, 10
N_CORES = 8
S = T // N_CORES
DC = D // 128
MT = 512

MM_DTYPE = "int8"
CLIP = 4.0
NI8 = 6  # int8 chunks 0..5 (5 DVE-cast + 1 ScalarE-cast)
NDV = 5  # of which DVE-cast
NBF = DC - NI8  # bf16 chunks 6,7
N_FILL = 14

_F32 = mybir.dt.float32
_BF16 = mybir.dt.bfloat16
_I8 = mybir.dt.int8

WB_COLS = DC * H + 2 + C  # w1 | b1(f32 as 2 bf16) | w2


def _chunks(total, step):
    out = []
    p = 0
    while p < total:
        c = min(step, total - p)
        out.append((p, c))
        p += c
    return out


def _unit_plan(M_task):
    subs = _chunks(M_task, MT)
    units = []
    i = 0
    for n in [1, 2, 2]:
        if i >= len(subs):
            break
        units.append(subs[i : i + n])
        i += n
    while i < len(subs):
        units.append(subs[i : i + 4])
        i += 4
    return units


def _relu_sched(units):
    """Greedy relu-engine assignment balancing measured per-op costs."""
    nsub = sum(len(u) for u in units) * S
    ncopy = sum(2 if any(w != MT for _, w in u) else 1 for u in units) * S
    load_v = nsub * (0.06 + NDV * 512 * 0.000521)
    load_a = nsub * (0.187 + 512 * 0.000833) + 6.8 + 0.6 * ncopy
    out = {}
    for ui in range(len(units)):
        for s in range(S):
            for j, (_, smt) in enumerate(units[ui]):
                dv = 0.691 * smt / 512
                da = 0.820 * smt / 512
                if load_v + dv <= load_a + da:
                    load_v += dv
                    out[(s, ui, j)] = "v"
                else:
                    load_a += da
                    out[(s, ui, j)] = "a"
    return out


def _build(M_task, mm_dtype=MM_DTYPE):
    assert mm_dtype == "int8"
    units = _unit_plan(M_task)
    NU = len(units)
    rsched = _relu_sched(units)

    nc = bacc.Bacc(None, target_bir_lowering=False)
    x8d = nc.declare_dram_parameter("x8d", [S, 128, NI8 * M_task], _I8, isOutput=False)
    xbd = nc.declare_dram_parameter("xbd", [S, 128, NBF * M_task], _BF16, isOutput=False)
    wb = nc.declare_dram_parameter("wb", [S, 128, WB_COLS], _BF16, isOutput=False)
    outP = nc.declare_dram_parameter("outP", [S, NU, 106, MT], _F32, isOutput=True)

    relu = mybir.ActivationFunctionType.Relu
    work = [(s, ui) for ui in range(NU) for s in range(S)]
    u_off = [0]
    for u in units:
        u_off.append(u_off[-1] + sum(w for _, w in u))
    # bf16 scalar-ring piece boundaries: [units0-1 | unit2 | unit3 ...]
    bf_cuts = sorted(set([0, u_off[min(2, NU)]] + u_off[3:]))
    bf_pieces = [
        (bf_cuts[i], bf_cuts[i + 1])
        for i in range(len(bf_cuts) - 1)
        if bf_cuts[i + 1] > bf_cuts[i]
    ]

    def bf_piece_of(m):
        for pi, (a, b) in enumerate(bf_pieces):
            if a <= m < b:
                return pi, a
        raise ValueError(m)

    with TileContext(nc) as tc:
        with (
            tc.tile_pool(name="wpool", bufs=1) as wpool,
            tc.tile_pool(name="x8pool", bufs=1) as x8pool,
            tc.tile_pool(name="xbfpool", bufs=1) as xbfpool,
            tc.tile_pool(name="xbbpool", bufs=5) as xbbpool,
            tc.tile_pool(name="xbcpool", bufs=4) as xbcpool,
            tc.tile_pool(name="hpool", bufs=4) as hpool,
            tc.tile_pool(name="opool", bufs=3) as opool,
            tc.tile_pool(name="warm", bufs=1) as warm,
            tc.tile_pool(name="psum1", bufs=5, space="PSUM") as psum1,
            tc.tile_pool(name="psum2", bufs=2, space="PSUM") as psum2,
            tc.tile_pool(name="psumw", bufs=1, space="PSUM") as psumw,
        ):  # PSUM banks: 5 + 2 + 1 = 8
            wsrc = warm.tile([128, 256], _F32, tag="wsrc")
            nc.gpsimd.memset(wsrc[:], 0.0)
            wv = wsrc[:].bitcast(_BF16)
            zcol = wsrc[:, 0:1]
            wps = psumw.tile([128, 256], _F32, tag="wps")
            for _ in range(N_FILL):
                nc.tensor.matmul(wps[:], wv[:, :128], wv[:, :256], start=True, stop=True)

            # scalar ring in first-need order
            wts = [None] * S
            xbf_t = {}

            def load_w(s):
                wbt = wpool.tile([128, WB_COLS], _BF16, tag=f"wb{s}", name=f"wb{s}")
                nc.scalar.dma_start(wbt, wb[s])
                w1t = wbt[:, : DC * H].rearrange("p (dc h) -> p dc h", dc=DC)
                b1t = wbt[:, DC * H : DC * H + 2].bitcast(_F32)
                w2t = wbt[:, DC * H + 2 :]
                wts[s] = (w1t, b1t, w2t)

            def load_bf(s, pi):
                a, b = bf_pieces[pi]
                t = xbfpool.tile(
                    [128, NBF * (b - a)], _BF16, tag=f"xbf{pi}_{s}", name=f"xbf{pi}_{s}"
                )
                nc.scalar.dma_start(t, xbd[s, :, NBF * a : NBF * b])
                xbf_t[(s, pi)] = t

            load_w(0)
            load_bf(0, 0)
            load_w(1)
            load_bf(1, 0)
            for pi in range(1, len(bf_pieces)):
                for s in range(S):
                    load_bf(s, pi)

            # int8 stream: per-(slot,unit) pieces; units 0-1 on sync ring,
            # units 2+ on SWDGE (raw SWDGE runs near line rate)
            x8_t = {}
            for ui in range(NU):
                for s in range(S):
                    w_u = u_off[ui + 1] - u_off[ui]
                    t = x8pool.tile(
                        [128, NI8 * w_u], _I8, tag=f"x8_{s}_{ui}", name=f"x8_{s}_{ui}"
                    )
                    eng = nc.sync if ui < 2 else nc.gpsimd
                    eng.dma_start(
                        t, x8d[s, :, NI8 * u_off[ui] : NI8 * u_off[ui + 1]]
                    )
                    x8_t[(s, ui)] = t

            outs = []
            for s, ui in work:
                w1t, b1t, w2t = wts[s]
                subs = units[ui]
                x8 = x8_t[(s, ui)]
                ps2 = psum2.tile([128, MT], _F32, tag="ps2")
                for j, (sm0, smt) in enumerate(subs):
                    so = NI8 * (sm0 - u_off[ui])
                    xbb = xbbpool.tile([128, NDV * smt], _BF16, tag="xbb")
                    nc.vector.tensor_copy(xbb, x8[:, so : so + NDV * smt])
                    xbc = xbcpool.tile([128, (NI8 - NDV) * smt], _BF16, tag="xbc")
                    nc.scalar.copy(xbc, x8[:, so + NDV * smt : so + NI8 * smt])
                    pi, pa = bf_piece_of(sm0)
                    bft = xbf_t[(s, pi)]
                    bfo = NBF * (sm0 - pa)
                    ps1 = psum1.tile([H, MT], _F32, tag="ps1")
                    for k in range(DC):
                        if k < NDV:
                            src = xbb[:, k * smt : (k + 1) * smt]
                        elif k < NI8:
                            ri = k - NDV
                            src = xbc[:, ri * smt : (ri + 1) * smt]
                        else:
                            ri = k - NI8
                            src = bft[:, bfo + ri * smt : bfo + (ri + 1) * smt]
                        nc.tensor.matmul(
                            ps1[:, :smt],
                            w1t[:, k, :],
                            src,
                            start=(k == 0),
                            stop=(k == DC - 1),
                        )
                    ht = hpool.tile([H, MT], _BF16, tag="h")
                    if rsched[(s, ui, j)] == "a":
                        nc.scalar.activation(ht[:, :smt], ps1[:, :smt], relu, bias=b1t)
                    else:
                        nc.vector.scalar_tensor_tensor(
                            ht[:, :smt],
                            ps1[:, :smt],
                            b1t,
                            zcol.to_broadcast([H, smt]),
                            mybir.AluOpType.add,
                            mybir.AluOpType.max,
                        )
                    nc.tensor.matmul(
                        ps2[32 * j : 32 * j + C, :smt],
                        w2t,
                        ht[:, :smt],
                        start=True,
                        stop=True,
                        skip_group_check=True,
                        tile_position=(0, 32 * j),
                    )
                # drain the unit's packed layer-2 bank; a ragged tail sub is
                # copied separately (its bank columns beyond its width were
                # never written)
                nfull = sum(1 for _, w in subs if w == MT)
                ot = opool.tile([106, MT], _F32, tag="o")
                if nfull:
                    npf = 32 * (nfull - 1) + C
                    nc.scalar.copy(ot[:npf, :], ps2[:npf, :])
                if nfull < len(subs):
                    jt = len(subs) - 1
                    wt = subs[jt][1]
                    nc.scalar.copy(
                        ot[32 * jt : 32 * jt + C, :wt], ps2[32 * jt : 32 * jt + C, :wt]
                    )
                outs.append((s, ui, ot))
            for s, ui, ot in outs:
                nc.gpsimd.dma_start(outP[s, ui], ot)
    nc.compile()
    return nc


def _prepare(x, task_id, W1, b1, W2, b2, mm_dtype=MM_DTYPE):
    assert mm_dtype == "int8"
    import ml_dtypes

    bf16 = np.dtype(ml_dtypes.bfloat16)
    x = np.ascontiguousarray(np.asarray(x, dtype=np.float32))
    task_id = np.asarray(task_id).astype(np.int64)
    W1 = np.asarray(W1, dtype=np.float32)
    b1 = np.asarray(b1, dtype=np.float32)
    W2 = np.asarray(W2, dtype=np.float32)

    scale = CLIP / 127.0
    xq_full = np.clip(np.rint(x * (1.0 / scale)), -127, 127).astype(np.int8)

    order = np.argsort(task_id, kind="stable")
    counts = np.bincount(task_id, minlength=T)
    starts = np.concatenate([[0], np.cumsum(counts)])
    M_task = max(128, int(-(-int(counts.max()) // 128) * 128))

    idx = np.zeros((T, M_task), dtype=np.int64)
    for t in range(T):
        idx[t, : counts[t]] = order[starts[t] : starts[t + 1]]

    W1s = (W1 * scale).astype(np.float32)
    units = _unit_plan(M_task)

    in_maps = []
    for c in range(N_CORES):
        ts_c = [S * c + s for s in range(S)]
        rows = idx[ts_c].reshape(-1)
        xg8 = xq_full[rows].reshape(S, M_task, D)
        # bf16 chunks multiply the host-prescaled W1 (x scale folded in),
        # so carry x/scale here to compensate
        xgf = x[rows].reshape(S, M_task, D) * np.float32(1.0 / scale)
        xc8 = xg8.reshape(S, M_task, DC, 128).transpose(0, 2, 3, 1)  # [S,c,p,m]
        xcf = xgf.reshape(S, M_task, DC, 128).transpose(0, 2, 3, 1)
        i8_parts = []
        bf_parts = []
        for u in units:
            for sm0, smt in u:
                i8_parts.append(
                    xc8[:, :NI8, :, sm0 : sm0 + smt]
                    .transpose(0, 2, 1, 3)
                    .reshape(S, 128, NI8 * smt)
                )
                bf_parts.append(
                    xcf[:, NI8:, :, sm0 : sm0 + smt]
                    .transpose(0, 2, 1, 3)
                    .reshape(S, 128, NBF * smt)
                )
        x8d = np.ascontiguousarray(np.concatenate(i8_parts, axis=2))
        xbd = np.ascontiguousarray(np.concatenate(bf_parts, axis=2)).astype(bf16)
        w1p = (
            W1s[ts_c]
            .reshape(S, DC, 128, H)
            .transpose(0, 2, 1, 3)
            .reshape(S, 128, DC * H)
            .astype(bf16)
        )
        b1cols = np.ascontiguousarray(b1[ts_c]).reshape(S, 128, 1).view(np.uint16)
        wblob = np.zeros((S, 128, WB_COLS), dtype=bf16)
        wblob[:, :, : DC * H] = w1p
        wblob[:, :, DC * H : DC * H + 2] = b1cols.view(bf16)
        wblob[:, :, DC * H + 2 :] = np.ascontiguousarray(W2[ts_c]).astype(bf16)
        in_maps.append({"x8d": x8d, "xbd": xbd, "wb": wblob})
    return in_maps, idx, counts, M_task


def _unshard(results, idx, counts, b_total=B, b2=None):
    M_task = idx.shape[1]
    units = _unit_plan(M_task)
    out = np.empty((b_total, C), dtype=np.float32)
    for c in range(N_CORES):
        yP = np.asarray(results[c]["outP"])  # [S, NU, 106, MT]
        for s in range(S):
            t = S * c + s
            cnt = counts[t]
            y = np.empty((M_task, C), dtype=np.float32)
            for ui, u in enumerate(units):
                for j, (sm0, smt) in enumerate(u):
                    y[sm0 : sm0 + smt] = yP[s, ui, 32 * j : 32 * j + C, :smt].T
            res = y[:cnt]
            if b2 is not None:
                res = res + b2[t]
            out[idx[t, :cnt]] = res
    return out


def kernel(x, task_id, W1, b1, W2, b2):
    b2 = np.asarray(b2, dtype=np.float32)
    in_maps, idx, counts, M_task = _prepare(x, task_id, W1, b1, W2, b2)
    nc = _build(M_task)
    try:
        res = run_bass_kernel_spmd(nc, in_maps, list(range(N_CORES)))
    except Exception:
        res = run_bass_kernel_spmd(nc, in_maps, list(range(N_CORES)))
    return _unshard(
        res.results, idx, counts, b_total=np.asarray(task_id).shape[0], b2=b2
    )


# revision 14
# speedup vs baseline: 1.3048x; 1.3048x over previous
"""MultiHeadClassifier (MoE routing) Trainium2 kernel — mixed-precision edition.

Problem: B=65536 samples of dim D=1024, each routed by task_id to one of
T=16 two-layer heads (D->H=128 relu -> C=10). Host routes samples to their
head (only ~17 GFLOP of useful work), data-parallel with 2 tasks per core
across 8 cores.

Per-core budget: the PE needs ~34us (bf16 streaming of ~8448 samples x
1024 contraction + layer 2), practical HBM share is ~320 GB/s (~33us for
10.5MB), and the int8->bf16 upconversion runs on DVE (2 elem/cyc) and
ScalarE (1 elem/cyc) whose combined capacity fits ~6 cast-chunks in the
PE window. All three constraints meet at a mixed-precision d-chunk split
(host-chosen, scale 4sigma/127 pre-folded into bf16 W1):
  - d-chunks 0..4: int8 on the sync HWDGE ring, DVE-cast per 512-col sub
  - d-chunk 5: int8, ScalarE-cast
  - d-chunk 6: fp8 e4m3 carrying x/scale — the PE consumes it DIRECTLY
    against bf16 weights (fp8 runs at bf16 speed; no cast, 1 byte)
  - d-chunk 7: bf16 carrying x/scale (no cast, 2 bytes)
  measured end-to-end rel err 1.35e-2 vs the 2e-2 gate.
Chunks 5-7 travel as ONE uint8 stream (4 bytes/column: int8|fp8|2xbf16,
bitcast-sliced on device) on the scalar ring behind the per-slot weight
blobs (b1 bitcast into bf16 columns), in first-need piece order
[w s0, u01 s0, w s1, u01 s1, u2 s0, u2 s1, u3 s0, ...]. relu+b1 and the
layer-2 PSUM->SBUF copy are split ScalarE/DVE by a greedy balancer; b2 is
added on the host during unshard; out-DMAs ride the idle SWDGE ring.

Every stream is packed *flat per partition* in sub-major order so each
DMA is one contiguous multi-KB run per partition (128 large descriptors)
and each engine cast is an exact contiguous 2D op. m-units per slot grow
[512, 1024, 1024, rest] so compute starts on the first ~0.5MB. All x
tiles stay resident in SBUF so no DMA instruction ever waits (a waiting
dma_start blocks its issuing engine's whole queue). PE warmup fillers
ride through the ~7us NEFF preamble so real matmuls start warm at 2.4GHz.
"""

import sys

import numpy as np

for _p in ("/opt/trn_rl_repo", "/root/.axon_site/_ro/trn_rl_repo"):
    if _p not in sys.path:
        sys.path.append(_p)

import concourse.bacc as bacc
import concourse.mybir as mybir
from concourse.bass_utils import run_bass_kernel_spmd
from concourse.tile import TileContext

B, D, T, H, C = 65536, 1024, 16, 128, 10
N_CORES = 8
S = T // N_CORES
DC = D // 128
MT = 512

MM_DTYPE = "int8"
CLIP = 4.0
NSY = 5  # int8 chunks 0..4 on sync ring, DVE-cast
N_FILL = 14

_F32 = mybir.dt.float32
_BF16 = mybir.dt.bfloat16
_I8 = mybir.dt.int8
_FP8 = mybir.dt.float8e4
_U8 = mybir.dt.uint8

WB_COLS = DC * H + 2 + C  # w1 | b1(f32 as 2 bf16) | w2


def _chunks(total, step):
    out = []
    p = 0
    while p < total:
        c = min(step, total - p)
        out.append((p, c))
        p += c
    return out


def _unit_plan(M_task):
    subs = _chunks(M_task, MT)
    units = []
    i = 0
    for n in [1, 2, 2]:
        if i >= len(subs):
            break
        units.append(subs[i : i + n])
        i += n
    while i < len(subs):
        units.append(subs[i : i + 4])
        i += 4
    return units


def _sched(units):
    """Greedy per-(slot,unit,sub) (relu_eng, copy_eng) assignment.

    Fixed loads (us): DVE 5-chunk casts; ScalarE 1-chunk casts + ~7us of
    scalar-ring DMA instruction time. Marginal costs from HW measurement.
    """
    nsub = sum(len(u) for u in units) * S
    load_v = nsub * (0.06 + NSY * 512 * 0.000521)
    load_a = nsub * (0.187 + 512 * 0.000833) + 7.0
    out = {}
    for ui in range(len(units)):
        for s in range(S):
            for j, (_, smt) in enumerate(units[ui]):
                f = smt / 512.0
                best = None
                for r in ("v", "a"):
                    for ce in ("v", "a"):
                        dv = (0.691 if r == "v" else 0.0) + (0.66 if ce == "v" else 0.0)
                        da = (0.820 if r == "a" else 0.0) + (0.57 if ce == "a" else 0.0)
                        m = max(load_v + dv * f, load_a + da * f)
                        if best is None or m < best[0]:
                            best = (m, r, ce, dv * f, da * f)
                _, r, ce, dv, da = best
                load_v += dv
                load_a += da
                out[(s, ui, j)] = (r, ce)
    return out


def _build(M_task, mm_dtype=MM_DTYPE):
    assert mm_dtype == "int8"
    units = _unit_plan(M_task)
    NU = len(units)
    sched = _sched(units)

    nc = bacc.Bacc(None, target_bir_lowering=False)
    x8d = nc.declare_dram_parameter("x8d", [S, 128, NSY * M_task], _I8, isOutput=False)
    # combined chunks 5-7 stream: per sub [int8 w | fp8 w | bf16 2w] bytes
    xcd = nc.declare_dram_parameter("xcd", [S, 128, 4 * M_task], _U8, isOutput=False)
    wb = nc.declare_dram_parameter("wb", [S, 128, WB_COLS], _BF16, isOutput=False)
    outT = nc.declare_dram_parameter("outT", [S, C, M_task], _F32, isOutput=True)

    relu = mybir.ActivationFunctionType.Relu
    work = [(s, ui) for ui in range(NU) for s in range(S)]
    u_off = [0]
    for u in units:
        u_off.append(u_off[-1] + sum(w for _, w in u))
    # scalar-ring piece boundaries for the combined stream
    cd_cuts = sorted(set([0, u_off[min(2, NU)]] + u_off[3:]))
    cd_pieces = [
        (cd_cuts[i], cd_cuts[i + 1])
        for i in range(len(cd_cuts) - 1)
        if cd_cuts[i + 1] > cd_cuts[i]
    ]

    def cd_piece_of(m):
        for pi, (a, b) in enumerate(cd_pieces):
            if a <= m < b:
                return pi, a
        raise ValueError(m)

    with TileContext(nc) as tc:
        with (
            tc.tile_pool(name="wpool", bufs=1) as wpool,
            tc.tile_pool(name="x8pool", bufs=1) as x8pool,
            tc.tile_pool(name="xcdpool", bufs=1) as xcdpool,
            tc.tile_pool(name="xbbpool", bufs=5) as xbbpool,
            tc.tile_pool(name="xbcpool", bufs=4) as xbcpool,
            tc.tile_pool(name="hpool", bufs=4) as hpool,
            tc.tile_pool(name="opool", bufs=3) as opool,
            tc.tile_pool(name="warm", bufs=1) as warm,
            tc.tile_pool(name="psum1", bufs=5, space="PSUM") as psum1,
            tc.tile_pool(name="psum2", bufs=2, space="PSUM") as psum2,
            tc.tile_pool(name="psumw", bufs=1, space="PSUM") as psumw,
        ):  # PSUM banks: 5 + 2 + 1 = 8
            wsrc = warm.tile([128, 256], _F32, tag="wsrc")
            nc.gpsimd.memset(wsrc[:], 0.0)
            wv = wsrc[:].bitcast(_BF16)
            zcol = wsrc[:, 0:1]
            wps = psumw.tile([128, 256], _F32, tag="wps")
            for _ in range(N_FILL):
                nc.tensor.matmul(wps[:], wv[:, :128], wv[:, :256], start=True, stop=True)

            # scalar ring in first-need order
            wts = [None] * S
            xcd_t = {}

            def load_w(s):
                wbt = wpool.tile([128, WB_COLS], _BF16, tag=f"wb{s}", name=f"wb{s}")
                nc.scalar.dma_start(wbt, wb[s])
                w1t = wbt[:, : DC * H].rearrange("p (dc h) -> p dc h", dc=DC)
                b1t = wbt[:, DC * H : DC * H + 2].bitcast(_F32)
                w2t = wbt[:, DC * H + 2 :]
                wts[s] = (w1t, b1t, w2t)

            def load_cd(s, pi):
                a, b = cd_pieces[pi]
                t = xcdpool.tile(
                    [128, 4 * (b - a)], _U8, tag=f"xcd{pi}_{s}", name=f"xcd{pi}_{s}"
                )
                nc.scalar.dma_start(t, xcd[s, :, 4 * a : 4 * b])
                xcd_t[(s, pi)] = t

            load_w(0)
            load_cd(0, 0)
            load_w(1)
            load_cd(1, 0)
            for pi in range(1, len(cd_pieces)):
                for s in range(S):
                    load_cd(s, pi)

            # sync ring: per-(slot,unit) int8 chunk 0-4 pieces
            x8_t = {}
            for ui in range(NU):
                for s in range(S):
                    w_u = u_off[ui + 1] - u_off[ui]
                    t = x8pool.tile(
                        [128, NSY * w_u], _I8, tag=f"x8_{s}_{ui}", name=f"x8_{s}_{ui}"
                    )
                    nc.sync.dma_start(
                        t, x8d[s, :, NSY * u_off[ui] : NSY * u_off[ui + 1]]
                    )
                    x8_t[(s, ui)] = t

            outs = []
            for s, ui in work:
                w1t, b1t, w2t = wts[s]
                subs = units[ui]
                x8 = x8_t[(s, ui)]
                ot = opool.tile(
                    [C, sum(w for _, w in subs)], _F32, tag="o", name=f"ot{s}_{ui}"
                )
                for j, (sm0, smt) in enumerate(subs):
                    r_eng, c_eng = sched[(s, ui, j)]
                    so = NSY * (sm0 - u_off[ui])
                    xbb = xbbpool.tile([128, NSY * smt], _BF16, tag="xbb")
                    nc.vector.tensor_copy(xbb, x8[:, so : so + NSY * smt])
                    pi, pa = cd_piece_of(sm0)
                    cdt = xcd_t[(s, pi)]
                    co = 4 * (sm0 - pa)
                    c5 = cdt[:, co : co + smt].bitcast(_I8)
                    c6 = cdt[:, co + smt : co + 2 * smt].bitcast(_FP8)
                    c7 = cdt[:, co + 2 * smt : co + 4 * smt].bitcast(_BF16)
                    xbc = xbcpool.tile([128, smt], _BF16, tag="xbc")
                    nc.scalar.copy(xbc, c5)
                    ps1 = psum1.tile([H, MT], _F32, tag="ps1")
                    for k in range(DC):
                        if k < NSY:
                            src = xbb[:, k * smt : (k + 1) * smt]
                        elif k == NSY:
                            src = xbc[:]
                        elif k == NSY + 1:
                            src = c6
                        else:
                            src = c7
                        nc.tensor.matmul(
                            ps1[:, :smt],
                            w1t[:, k, :],
                            src,
                            start=(k == 0),
                            stop=(k == DC - 1),
                        )
                    ht = hpool.tile([H, MT], _BF16, tag="h")
                    if r_eng == "a":
                        nc.scalar.activation(ht[:, :smt], ps1[:, :smt], relu, bias=b1t)
                    else:
                        nc.vector.scalar_tensor_tensor(
                            ht[:, :smt],
                            ps1[:, :smt],
                            b1t,
                            zcol.to_broadcast([H, smt]),
                            mybir.AluOpType.add,
                            mybir.AluOpType.max,
                        )
                    ps2 = psum2.tile([C, MT], _F32, tag="ps2")
                    nc.tensor.matmul(
                        ps2[:, :smt], w2t, ht[:, :smt], start=True, stop=True
                    )
                    dst = ot[:, sm0 - u_off[ui] : sm0 - u_off[ui] + smt]
                    if c_eng == "a":
                        nc.scalar.copy(dst, ps2[:, :smt])
                    else:
                        nc.vector.tensor_copy(dst, ps2[:, :smt])
                outs.append((s, u_off[ui], u_off[ui + 1] - u_off[ui], ot))
            for s, m0, w_u, ot in outs:
                nc.gpsimd.dma_start(outT[s, :, m0 : m0 + w_u], ot)
    nc.compile()
    return nc


def _prepare(x, task_id, W1, b1, W2, b2, mm_dtype=MM_DTYPE):
    assert mm_dtype == "int8"
    import ml_dtypes

    bf16 = np.dtype(ml_dtypes.bfloat16)
    fp8 = np.dtype(ml_dtypes.float8_e4m3fn)
    x = np.ascontiguousarray(np.asarray(x, dtype=np.float32))
    task_id = np.asarray(task_id).astype(np.int64)
    W1 = np.asarray(W1, dtype=np.float32)
    b1 = np.asarray(b1, dtype=np.float32)
    W2 = np.asarray(W2, dtype=np.float32)

    scale = CLIP / 127.0
    xq_full = np.clip(np.rint(x * (1.0 / scale)), -127, 127).astype(np.int8)

    order = np.argsort(task_id, kind="stable")
    counts = np.bincount(task_id, minlength=T)
    starts = np.concatenate([[0], np.cumsum(counts)])
    M_task = max(128, int(-(-int(counts.max()) // 128) * 128))

    idx = np.zeros((T, M_task), dtype=np.int64)
    for t in range(T):
        idx[t, : counts[t]] = order[starts[t] : starts[t + 1]]

    W1s = (W1 * scale).astype(np.float32)
    units = _unit_plan(M_task)

    in_maps = []
    for c in range(N_CORES):
        ts_c = [S * c + s for s in range(S)]
        rows = idx[ts_c].reshape(-1)
        xg8 = xq_full[rows].reshape(S, M_task, D)
        # chunks 6,7 multiply the host-prescaled W1, so they carry x/scale
        xgf = x[rows].reshape(S, M_task, D) * np.float32(1.0 / scale)
        xc8 = xg8.reshape(S, M_task, DC, 128).transpose(0, 2, 3, 1)  # [S,c,p,m]
        xcf = xgf.reshape(S, M_task, DC, 128).transpose(0, 2, 3, 1)
        i8_parts = []
        cd_parts = []
        for u in units:
            for sm0, smt in u:
                i8_parts.append(
                    xc8[:, :NSY, :, sm0 : sm0 + smt]
                    .transpose(0, 2, 1, 3)
                    .reshape(S, 128, NSY * smt)
                )
                p5 = np.ascontiguousarray(xc8[:, NSY, :, sm0 : sm0 + smt]).view(
                    np.uint8
                )  # [S,128,w]
                p6 = (
                    np.ascontiguousarray(xcf[:, NSY + 1, :, sm0 : sm0 + smt])
                    .astype(fp8)
                    .view(np.uint8)
                )  # [S,128,w]
                p7 = (
                    np.ascontiguousarray(xcf[:, NSY + 2, :, sm0 : sm0 + smt])
                    .astype(bf16)
                    .view(np.uint8)
                )  # [S,128,2w]
                cd_parts.append(np.concatenate([p5, p6, p7], axis=2))
        x8d = np.ascontiguousarray(np.concatenate(i8_parts, axis=2))
        xcd = np.ascontiguousarray(np.concatenate(cd_parts, axis=2))
        w1p = (
            W1s[ts_c]
            .reshape(S, DC, 128, H)
            .transpose(0, 2, 1, 3)
            .reshape(S, 128, DC * H)
            .astype(bf16)
        )
        b1cols = np.ascontiguousarray(b1[ts_c]).reshape(S, 128, 1).view(np.uint16)
        wblob = np.zeros((S, 128, WB_COLS), dtype=bf16)
        wblob[:, :, : DC * H] = w1p
        wblob[:, :, DC * H : DC * H + 2] = b1cols.view(bf16)
        wblob[:, :, DC * H + 2 :] = np.ascontiguousarray(W2[ts_c]).astype(bf16)
        in_maps.append({"x8d": x8d, "xcd": xcd, "wb": wblob})
    return in_maps, idx, counts, M_task


def _unshard(results, idx, counts, b_total=B, b2=None):
    out = np.empty((b_total, C), dtype=np.float32)
    for c in range(N_CORES):
        yT = np.asarray(results[c]["outT"])  # [S, C, M_task]
        y = yT.transpose(0, 2, 1)
        for s in range(S):
            t = S * c + s
            cnt = counts[t]
            res = y[s, :cnt]
            if b2 is not None:
                res = res + b2[t]
            out[idx[t, :cnt]] = res
    return out


def kernel(x, task_id, W1, b1, W2, b2):
    b2 = np.asarray(b2, dtype=np.float32)
    in_maps, idx, counts, M_task = _prepare(x, task_id, W1, b1, W2, b2)
    nc = _build(M_task)
    try:
        res = run_bass_kernel_spmd(nc, in_maps, list(range(N_CORES)))
    except Exception:
        res = run_bass_kernel_spmd(nc, in_maps, list(range(N_CORES)))
    return _unshard(
        res.results, idx, counts, b_total=np.asarray(task_id).shape[0], b2=b2
    )
